# revision 5
# baseline (speedup 1.0000x reference)
"""Trainium2 Bass kernel for a 2-layer post-LN decoder (nn_Decoder_54082228191615).

Sharding: 8 cores = 2 batch groups x 4 query-slice ranks.
Core (b, r) computes all 12 heads for query tokens [512r, 512r+512) of batch b,
writes its attn-prob slice transposed ([tk, tq_local], bf16), and its slice of
the residual stream. One bf16 AllGather per batch group provides the full
layer-1 output for layer-2 K/V. Host reassembles/transposes the full outputs.

All on-chip activations are kept transposed [feature, token] so matmuls need
no on-chip transposes; softmax denominators come from ones-vector matmuls.
Heads are packed in pairs onto the 128-wide partition dim (tile_position).
"""

import numpy as np
import ml_dtypes

import concourse.bass as bass
import concourse.tile as tile
from concourse import bacc, mybir
from concourse.bass_utils import run_bass_kernel_spmd
from concourse.masks import make_identity

F32 = mybir.dt.float32
BF16 = mybir.dt.bfloat16
I32 = mybir.dt.int32
AF = mybir.ActivationFunctionType
ALU = mybir.AluOpType

B, S, D, H, DK, DFF, L = 2, 2048, 768, 12, 64, 3072, 2
VOCAB = 32000
P = 128
DC = D // P            # 6 feature chunks
TC = S // P            # 16 token chunks
Q = 512                # local query-slice width
QC = Q // P            # 4 local token chunks
FC = DFF // P          # 24 ffn chunks
HC = H // 2            # 6 head pairs
N_CORES = 8
GROUP = 4
EPS = 1e-5
SCALE = 1.0 / 8.0      # 1/sqrt(DK)

_CACHED = {}


def _layernorm(nc, pools, z, ln_sb, gcol, bcol, ones_f32, nm):
    """Post-LN over the feature (partition) dim of z: [6][128, Q] f32.
    Destroys z in place. Returns (x_f32, x_bf) lists [6][128,Q]."""
    sb, ps, small = pools["sb"], pools["ps"], pools["small"]
    m_ps = ps.tile([1, Q], F32, space="PSUM", name=f"mps_{nm}", tag="aux", bufs=2)
    s_ps = ps.tile([1, Q], F32, space="PSUM", name=f"ssps_{nm}", tag="aux", bufs=2)
    for dc in range(DC):
        sqt = sb.tile([P, Q], F32, name=f"sq_{nm}_{dc}", tag="lntmp", bufs=2)
        nc.scalar.activation(sqt[:], z[dc][:], AF.Square)
        nc.tensor.matmul(m_ps[:], ones_f32[:, 0:1], z[dc][:],
                         start=(dc == 0), stop=(dc == DC - 1))
        nc.tensor.matmul(s_ps[:], ones_f32[:, 0:1], sqt[:],
                         start=(dc == 0), stop=(dc == DC - 1))
    m = small.tile([1, Q], F32, name=f"m_{nm}", tag="m", bufs=1)
    nc.vector.tensor_scalar_mul(m[:], m_ps[:], 1.0 / D)
    msq = small.tile([1, Q], F32, name=f"msq_{nm}", tag="msq", bufs=1)
    nc.vector.tensor_scalar_mul(msq[:], s_ps[:], 1.0 / D)
    m2 = small.tile([1, Q], F32, name=f"m2_{nm}", tag="m2", bufs=1)
    nc.vector.tensor_tensor(m2[:], m[:], m[:], op=ALU.mult)
    # var -> msq (in place), std -> m2 (in place), rstd -> msq
    nc.vector.tensor_sub(msq[:], msq[:], m2[:])
    nc.scalar.activation(m2[:], msq[:], AF.Sqrt, bias=pools["eps"][0:1, 0:1])
    nc.vector.reciprocal(msq[:], m2[:])
    mb = sb.tile([P, Q], F32, name=f"mb_{nm}", tag="lntmp", bufs=2)
    nc.gpsimd.partition_broadcast(mb[:], m[:])
    rstdb = sb.tile([P, Q], F32, name=f"rstdb_{nm}", tag="lntmp", bufs=2)
    nc.gpsimd.partition_broadcast(rstdb[:], msq[:])
    xf, xb = [], []
    for dc in range(DC):
        nc.vector.tensor_sub(z[dc][:], z[dc][:], mb[:])
        nc.vector.tensor_tensor(z[dc][:], z[dc][:], rstdb[:], op=ALU.mult)
        xft = sb.tile([P, Q], F32, name=f"xf_{nm}_{dc}", tag=f"xout{dc}", bufs=2)
        nc.scalar.activation(xft[:], z[dc][:], AF.Identity,
                             scale=ln_sb[dc][:, gcol:gcol + 1],
                             bias=ln_sb[dc][:, bcol:bcol + 1])
        xbt = sb.tile([P, Q], BF16, name=f"xb_{nm}_{dc}", tag=f"xoutb{dc}", bufs=2)
        nc.vector.tensor_copy(xbt[:], xft[:])
        xf.append(xft)
        xb.append(xbt)
    return xf, xb


def _build_layer(nc, pools, dtens, l, xfull_bf, xq_bf, x_resid, ones_bf, ones_f32,
                 keep_sb, ln_sb):
    """Emit one decoder layer. x_resid is consumed in place.
    Returns (x_f32, x_bf) lists of [6][128,Q]."""
    sb, ps, small = pools["sb"], pools["ps"], pools["small"]
    wq_t, wk_t, wv_t, wo_t, w1_t, w2_t, b1_t, b2_t, attn_out = dtens

    b1_sb = sb.tile([P, FC], F32, name=f"b1sb{l}", tag="b1", bufs=2)
    nc.sync.dma_start(b1_sb[:],
                      b1_t.ap().rearrange("l (c p) -> p l c", p=P)[:, l, :])
    b2_sb = sb.tile([P, DC], F32, name=f"b2sb{l}", tag="b2", bufs=2)
    nc.sync.dma_start(b2_sb[:],
                      b2_t.ap().rearrange("l (c p) -> p l c", p=P)[:, l, :])

    def load_w(dram_t, nm):
        t = sb.tile([P, DC, D], BF16, name=nm, tag="wbig", bufs=1)
        nc.gpsimd.dma_start(t[:],
                            dram_t.ap()[l].rearrange("(dc p) f -> p dc f", p=P))
        return t

    # ---- q projection (pair-packed: psum halves via tile_position) ----
    wq_sb = load_w(wq_t, f"wq{l}")
    qT2 = []
    for hc in range(HC):
        q_ps = ps.tile([P, Q], F32, space="PSUM", name=f"qps{l}_{hc}", tag="proj",
                       bufs=2)
        for sub in range(2):
            h = 2 * hc + sub
            for dc in range(DC):
                nc.tensor.matmul(q_ps[64 * sub:64 * sub + 64, :],
                                 wq_sb[:, dc, DK * h:DK * (h + 1)], xq_bf[dc][:],
                                 start=(dc == 0), stop=(dc == DC - 1),
                                 tile_position=(0, 64 * sub))
        qh = sb.tile([P, Q], BF16, name=f"qT{l}_{hc}", tag=f"qT{hc}", bufs=1)
        nc.scalar.copy(qh[:], q_ps[:])
        qT2.append(qh)

    # ---- k projection ----
    wk_sb = load_w(wk_t, f"wk{l}")
    kT2 = []
    for hc in range(HC):
        kh = sb.tile([P, S], BF16, name=f"kT{l}_{hc}", tag=f"kT{hc}", bufs=1)
        for t4 in range(S // 512):
            k_ps = ps.tile([P, 512], F32, space="PSUM", name=f"kps{l}_{hc}_{t4}",
                           tag="proj", bufs=2)
            for sub in range(2):
                h = 2 * hc + sub
                for dc in range(DC):
                    nc.tensor.matmul(k_ps[64 * sub:64 * sub + 64, :],
                                     wk_sb[:, dc, DK * h:DK * (h + 1)],
                                     xfull_bf[dc][:, 512 * t4:512 * (t4 + 1)],
                                     start=(dc == 0), stop=(dc == DC - 1),
                                     tile_position=(0, 64 * sub))
            nc.scalar.copy(kh[:, 512 * t4:512 * (t4 + 1)], k_ps[:])
        kT2.append(kh)

    # ---- v natural per token chunk: v_sb[c] [128, H, 64] bf16 ----
    wv_sb = load_w(wv_t, f"wv{l}")
    v_sb = []
    for c in range(TC):
        vt = sb.tile([P, H, DK], BF16, name=f"v{l}_{c}", tag=f"v{c}", bufs=1)
        for half in range(2):
            v_ps = ps.tile([P, 384], F32, space="PSUM", name=f"vps{l}_{c}_{half}",
                           tag="proj", bufs=2)
            for dc in range(DC):
                nc.tensor.matmul(v_ps[:], xfull_bf[dc][:, P * c:P * (c + 1)],
                                 wv_sb[:, dc, 384 * half:384 * (half + 1)],
                                 start=(dc == 0), stop=(dc == DC - 1))
            nc.vector.tensor_copy(vt[:, 6 * half:6 * (half + 1), :], v_ps[:])
        v_sb.append(vt)

    # ---- attention, head pairs ----
    ctxT2 = []
    for hc in range(HC):
        ch = sb.tile([P, Q], BF16, name=f"ctxT{l}_{hc}", tag=f"ctxT{hc}", bufs=1)
        ctx_ps = ps.tile([P, Q], F32, space="PSUM", name=f"ctxps{l}_{hc}",
                         tag="ctx", bufs=1)
        for sub in range(2):
            h = 2 * hc + sub
            pr = slice(64 * sub, 64 * sub + 64)
            sum_ps = ps.tile([1, Q], F32, space="PSUM", name=f"sumps{l}_{h}",
                             tag="aux", bufs=2)
            um = []
            for c in range(TC):
                s_ps = ps.tile([P, Q], F32, space="PSUM", name=f"sps{l}_{h}_{c}",
                               tag="scores", bufs=2)
                nc.tensor.matmul(s_ps[:], kT2[hc][pr, P * c:P * (c + 1)],
                                 qT2[hc][pr, :], start=True, stop=True)
                um_c = sb.tile([P, Q], BF16, name=f"um{l}_{h}_{c}", tag=f"um{c}",
                               bufs=1)
                nc.scalar.activation(um_c[:], s_ps[:], AF.Exp, scale=SCALE)
                nc.vector.tensor_tensor(um_c[:], um_c[:], keep_sb[c][:],
                                        op=ALU.mult)
                nc.tensor.matmul(ctx_ps[pr, :], v_sb[c][:, h, :], um_c[:],
                                 start=(c == 0), stop=(c == TC - 1),
                                 tile_position=(0, 64 * sub))
                nc.tensor.matmul(sum_ps[:], ones_bf[:, 0:1], um_c[:],
                                 start=(c == 0), stop=(c == TC - 1))
                um.append(um_c)
            recip = small.tile([1, Q], F32, name=f"recip{l}_{h}", tag="recip",
                               bufs=2)
            nc.vector.reciprocal(recip[:], sum_ps[:])
            rb = sb.tile([P, Q], F32, name=f"rb{l}_{h}", tag="rb", bufs=2)
            nc.gpsimd.partition_broadcast(rb[:], recip[:])
            for c in range(TC):
                at = sb.tile([P, Q], BF16, name=f"at{l}_{h}_{c}", tag="attnstage",
                             bufs=2)
                nc.vector.tensor_tensor(at[:], um[c][:], rb[:], op=ALU.mult)
                nc.sync.dma_start(attn_out.ap()[l, h, c], at[:])
            nc.vector.tensor_tensor(ch[pr, :], ctx_ps[pr, :], rb[pr, :],
                                    op=ALU.mult)
        ctxT2.append(ch)

    # ---- wo + residual (in-place into x_resid) + LN1 ----
    wo_sb = load_w(wo_t, f"wo{l}")
    for dc in range(DC):
        o_ps = ps.tile([P, Q], F32, space="PSUM", name=f"ops{l}_{dc}", tag="proj",
                       bufs=2)
        for hc in range(HC):
            nc.tensor.matmul(o_ps[:], wo_sb[:, hc, P * dc:P * (dc + 1)],
                             ctxT2[hc][:], start=(hc == 0), stop=(hc == HC - 1))
        nc.vector.tensor_add(x_resid[dc][:], o_ps[:], x_resid[dc][:])
    x1f, x1b = _layernorm(nc, pools, x_resid, ln_sb, 4 * l + 0, 4 * l + 1,
                          ones_f32, f"ln1_{l}")

    # ---- FFN: two dc-groups of 3, hidden tiles transient ----
    z2 = []
    for dcg in range(2):
        dcs = [3 * dcg + j for j in range(3)]
        f_tags = ["scores", "scores", "ctx"]
        f_ps = [ps.tile([P, Q], F32, space="PSUM", name=f"fps{l}_{dc}",
                        tag=f_tags[j], bufs=None if f_tags[j] == "ctx" else 2)
                for j, dc in enumerate(dcs)]
        for fc in range(FC):
            w1t = sb.tile([P, DC, P], BF16, name=f"w1_{l}_{dcg}_{fc}", tag="w1s",
                          bufs=2)
            nc.gpsimd.dma_start(
                w1t[:],
                w1_t.ap()[l].rearrange("(dc p) f -> p dc f",
                                       p=P)[:, :, P * fc:P * (fc + 1)])
            h_ps = ps.tile([P, Q], F32, space="PSUM", name=f"hps{l}_{dcg}_{fc}",
                           tag="proj", bufs=2)
            for dc in range(DC):
                nc.tensor.matmul(h_ps[:], w1t[:, dc, :], x1b[dc][:],
                                 start=(dc == 0), stop=(dc == DC - 1))
            ht = sb.tile([P, Q], BF16, name=f"hT{l}_{dcg}_{fc}",
                         tag=f"um{fc % 4}", bufs=1)
            nc.scalar.activation(ht[:], h_ps[:], AF.Relu,
                                 bias=b1_sb[:, fc:fc + 1])
            w2t = sb.tile([P, 3, P], BF16, name=f"w2_{l}_{dcg}_{fc}", tag="w2s",
                          bufs=3)
            nc.gpsimd.dma_start(
                w2t[:],
                w2_t.ap()[l].rearrange("(fc p) f -> p fc f",
                                       p=P)[:, fc, 384 * dcg:384 * (dcg + 1)]
                .rearrange("p (c q) -> p c q", q=P))
            for j in range(3):
                nc.tensor.matmul(f_ps[j][:], w2t[:, j, :], ht[:],
                                 start=(fc == 0), stop=(fc == FC - 1))
        for j, dc in enumerate(dcs):
            zt = sb.tile([P, Q], F32, name=f"z2_{l}_{dc}", tag=f"resid{dc}",
                         bufs=1)
            nc.scalar.activation(zt[:], f_ps[j][:], AF.Identity,
                                 bias=b2_sb[:, dc:dc + 1])
            nc.vector.tensor_add(zt[:], zt[:], x1f[dc][:])
            z2.append(zt)
    return _layernorm(nc, pools, z2, ln_sb, 4 * l + 2, 4 * l + 3, ones_f32,
                      f"ln2_{l}")


def build_program():
    nc = bacc.Bacc("TRN2", target_bir_lowering=False, debug=False,
                   enable_asserts=False, num_devices=N_CORES)

    emb_t = nc.dram_tensor("emb", [VOCAB, D], F32, kind="ExternalInput")
    pos_full_t = nc.dram_tensor("pos_full", [S, D], F32, kind="ExternalInput")
    pos_loc_t = nc.dram_tensor("pos_loc", [Q, D], F32, kind="ExternalInput")
    idx_full_t = nc.dram_tensor("idx_full", [TC, P, 1], I32, kind="ExternalInput")
    idx_loc_t = nc.dram_tensor("idx_loc", [QC, P, 1], I32, kind="ExternalInput")
    keep_t = nc.dram_tensor("keep", [TC, P, Q], BF16, kind="ExternalInput")
    wq_t = nc.dram_tensor("wq", [L, D, D], F32, kind="ExternalInput")
    wk_t = nc.dram_tensor("wk", [L, D, D], F32, kind="ExternalInput")
    wv_t = nc.dram_tensor("wv", [L, D, D], F32, kind="ExternalInput")
    wo_t = nc.dram_tensor("wo", [L, D, D], F32, kind="ExternalInput")
    w1_t = nc.dram_tensor("w1", [L, D, DFF], F32, kind="ExternalInput")
    w2_t = nc.dram_tensor("w2", [L, DFF, D], F32, kind="ExternalInput")
    b1_t = nc.dram_tensor("b1", [L, DFF], F32, kind="ExternalInput")
    b2_t = nc.dram_tensor("b2", [L, D], F32, kind="ExternalInput")
    ln_t = nc.dram_tensor("lnp", [L * 4, D], F32, kind="ExternalInput")

    attn_out = nc.dram_tensor("attn_out", [L, H, TC, P, Q], BF16,
                              kind="ExternalOutput")
    x_out = nc.dram_tensor("x_out", [DC, P, Q], F32, kind="ExternalOutput")

    with tile.TileContext(nc, num_cores=N_CORES) as tc:
        with tc.tile_pool(name="sb", bufs=1) as sb, \
             tc.tile_pool(name="small", bufs=1) as small, \
             tc.tile_pool(name="ps", bufs=1, space="PSUM") as ps, \
             tc.tile_pool(name="dram", bufs=1, space="DRAM") as dram:
            pools = dict(sb=sb, ps=ps, small=small)

            ident = sb.tile([P, P], F32, name="ident", tag="ident", bufs=1)
            make_identity(nc, ident[:])
            ones_bf = sb.tile([P, 1], BF16, name="ones_bf", tag="ones_bf", bufs=1)
            nc.gpsimd.memset(ones_bf[:], 1.0)
            ones_f32 = sb.tile([P, 1], F32, name="ones_f32", tag="ones_f32",
                               bufs=1)
            nc.gpsimd.memset(ones_f32[:], 1.0)
            eps_sb = small.tile([1, 1], F32, name="eps_sb", tag="eps", bufs=1)
            nc.gpsimd.memset(eps_sb[:], EPS)
            pools["eps"] = eps_sb
            ln_sb = []
            for dc in range(DC):
                t = sb.tile([P, L * 4], F32, name=f"ln_sb{dc}", tag=f"ln{dc}",
                            bufs=1)
                nc.sync.dma_start(
                    t[:], ln_t.ap().rearrange("r (c p) -> p r c", p=P)[:, :, dc])
                ln_sb.append(t)
            keep_sb = []
            for c in range(TC):
                t = sb.tile([P, Q], BF16, name=f"keep{c}", tag=f"keep{c}", bufs=1)
                nc.sync.dma_start(t[:], keep_t.ap()[c])
                keep_sb.append(t)

            x1_bf = [sb.tile([P, S], BF16, name=f"x1bf{dc}", tag=f"xfull{dc}",
                             bufs=1) for dc in range(DC)]
            x_resid = [sb.tile([P, Q], F32, name=f"xres{dc}", tag=f"resid{dc}",
                               bufs=1) for dc in range(DC)]
            xq_bf = [sb.tile([P, Q], BF16, name=f"xqbf{dc}", tag=f"xoutb{dc}",
                             bufs=2) for dc in range(DC)]

            def gather_chunk(idx_ap, pos_ap, c, dsts):
                idx_sb = sb.tile([P, 1], I32, name=f"idx{c}", tag="idx", bufs=2)
                nc.sync.dma_start(idx_sb[:], idx_ap)
                xn = sb.tile([P, D], F32, name=f"xn{c}", tag="nat", bufs=2)
                nc.gpsimd.indirect_dma_start(
                    out=xn[:], out_offset=None, in_=emb_t.ap(),
                    in_offset=bass.IndirectOffsetOnAxis(ap=idx_sb[:, 0:1], axis=0))
                pn = sb.tile([P, D], F32, name=f"pn{c}", tag="nat", bufs=2)
                nc.sync.dma_start(pn[:], pos_ap)
                nc.vector.tensor_add(xn[:], xn[:], pn[:])
                for dc in range(DC):
                    t_ps = ps.tile([P, P], F32, space="PSUM", name=f"tp{c}_{dc}",
                                   tag="aux", bufs=2)
                    nc.tensor.transpose(t_ps[:], xn[:, P * dc:P * (dc + 1)],
                                        ident[:])
                    dsts(dc, t_ps)

            for c in range(TC):
                def dst_full(dc, t_ps, c=c):
                    nc.scalar.copy(x1_bf[dc][:, P * c:P * (c + 1)], t_ps[:])
                gather_chunk(idx_full_t.ap()[c],
                             pos_full_t.ap()[P * c:P * (c + 1), :], c, dst_full)
            for c in range(QC):
                def dst_loc(dc, t_ps, c=c):
                    nc.vector.tensor_copy(x_resid[dc][:, P * c:P * (c + 1)],
                                          t_ps[:])
                    nc.scalar.copy(xq_bf[dc][:, P * c:P * (c + 1)], t_ps[:])
                gather_chunk(idx_loc_t.ap()[c],
                             pos_loc_t.ap()[P * c:P * (c + 1), :], TC + c,
                             dst_loc)

            dtens = (wq_t, wk_t, wv_t, wo_t, w1_t, w2_t, b1_t, b2_t, attn_out)

            x2f, x2b = _build_layer(nc, pools, dtens, 0, x1_bf, xq_bf, x_resid,
                                    ones_bf, ones_f32, keep_sb, ln_sb)

            cc_in = dram.tile([DC, P, Q], BF16, name="cc_in", tag="cc_in")
            for dc in range(DC):
                nc.sync.dma_start(cc_in[dc], x2b[dc][:])
            cc_out = dram.tile([GROUP, DC, P, Q], BF16, name="cc_out",
                               tag="cc_out")
            nc.gpsimd.collective_compute(
                "AllGather", ALU.bypass,
                replica_groups=[[0, 1, 2, 3], [4, 5, 6, 7]],
                ins=[cc_in[:]], outs=[cc_out[:]])
            x2_full = [sb.tile([P, S], BF16, name=f"x2bf{dc}", tag=f"xfull{dc}",
                               bufs=1) for dc in range(DC)]
            for g in range(GROUP):
                for dc in range(DC):
                    nc.sync.dma_start(x2_full[dc][:, Q * g:Q * (g + 1)],
                                      cc_out[g, dc])

            x3f, _x3b = _build_layer(nc, pools, dtens, 1, x2_full, x2b, x2f,
                                     ones_bf, ones_f32, keep_sb, ln_sb)

            for dc in range(DC):
                nc.sync.dma_start(x_out.ap()[dc], x3f[dc][:])

    nc.compile()
    return nc


def _get_program():
    if "nc" not in _CACHED:
        _CACHED["nc"] = build_program()
    return _CACHED["nc"]


def _prep_in_maps(inputs):
    idx = np.asarray(inputs["inputs"]).astype(np.int32)            # [B, S]
    amask = np.asarray(inputs["attention_mask"]).astype(np.int32)  # [B, S]
    pos = np.ascontiguousarray(np.asarray(inputs["pos_emb"], np.float32)[:S])
    lnp = np.stack([inputs["ln1_g"], inputs["ln1_b"],
                    inputs["ln2_g"], inputs["ln2_b"]], axis=1)     # [L, 4, D]
    lnp = np.ascontiguousarray(lnp.reshape(L * 4, D).astype(np.float32))

    shared = {
        "emb": np.ascontiguousarray(np.asarray(inputs["emb"], np.float32)),
        "pos_full": pos,
        "wq": np.ascontiguousarray(np.asarray(inputs["wq"], np.float32)),
        "wk": np.ascontiguousarray(np.asarray(inputs["wk"], np.float32)),
        "wv": np.ascontiguousarray(np.asarray(inputs["wv"], np.float32)),
        "wo": np.ascontiguousarray(np.asarray(inputs["wo"], np.float32)),
        "w1": np.ascontiguousarray(np.asarray(inputs["w1"], np.float32)),
        "w2": np.ascontiguousarray(np.asarray(inputs["w2"], np.float32)),
        "b1": np.ascontiguousarray(np.asarray(inputs["b1"], np.float32)),
        "b2": np.ascontiguousarray(np.asarray(inputs["b2"], np.float32)),
        "lnp": lnp,
    }
    tk = np.arange(S)[:, None]                     # [S, 1] key positions
    in_maps = []
    for core in range(N_CORES):
        b, r = divmod(core, GROUP)
        o = Q * r
        tq = np.arange(o, o + Q)[None, :]          # [1, Q] query positions
        pad = (amask[b] == 0)[:, None]             # [S, 1]
        keep = (pad | (tk > tq)).astype(np.float32)  # [S, Q]
        m = dict(shared)
        m["pos_loc"] = np.ascontiguousarray(pos[o:o + Q])
        m["idx_full"] = np.ascontiguousarray(idx[b].reshape(TC, P, 1))
        m["idx_loc"] = np.ascontiguousarray(idx[b, o:o + Q].reshape(QC, P, 1))
        m["keep"] = np.ascontiguousarray(
            keep.reshape(TC, P, Q).astype(ml_dtypes.bfloat16))
        in_maps.append(m)
    return in_maps


def kernel(**inputs):
    nc = _get_program()
    in_maps = _prep_in_maps(inputs)
    res = run_bass_kernel_spmd(nc, in_maps, core_ids=list(range(N_CORES)))
    x = np.zeros((B, S, D), np.float32)
    attns = np.zeros((L, B, H, S, S), np.float32)
    for core in range(N_CORES):
        b, r = divmod(core, GROUP)
        o = Q * r
        out = res.results[core]
        x[b, o:o + Q, :] = out["x_out"].reshape(D, Q).T
        a = out["attn_out"].reshape(L, H, S, Q).astype(np.float32)
        attns[:, b, :, o:o + Q, :] = a.transpose(0, 1, 3, 2)
    return x, attns


if __name__ == "__main__":
    _get_program()
    print("program built ok")


# revision 12
# speedup vs baseline: 1.0809x; 1.0809x over previous
"""Trainium2 Bass kernel for a 2-layer post-LN decoder (nn_Decoder_54082228191615).

Sharding: 8 cores = 2 batch groups x 4 query-slice ranks.
Core (b, r) computes all 12 heads for query tokens [512r, 512r+512) of batch b,
writes its attn-prob slice transposed ([tk, tq_local], bf16), and its slice of
the residual stream. One bf16 AllGather per batch group provides the full
layer-1 output for layer-2 K/V. Host reassembles/transposes the full outputs.

All on-chip activations are kept transposed [feature, token] so matmuls need
no on-chip transposes; softmax denominators come from ones-vector matmuls.
Heads are packed in pairs onto the 128-wide partition dim (tile_position).
"""

import numpy as np
import ml_dtypes

import concourse.bass as bass
import concourse.tile as tile
from concourse import bacc, mybir
from concourse.bass_utils import run_bass_kernel_spmd
from concourse.masks import make_identity

F32 = mybir.dt.float32
BF16 = mybir.dt.bfloat16
I32 = mybir.dt.int32
AF = mybir.ActivationFunctionType
ALU = mybir.AluOpType

B, S, D, H, DK, DFF, L = 2, 2048, 768, 12, 64, 3072, 2
VOCAB = 32000
P = 128
DC = D // P            # 6 feature chunks
TC = S // P            # 16 token chunks
Q = 512                # local query-slice width
QC = Q // P            # 4 local token chunks
FC = DFF // P          # 24 ffn chunks
HC = H // 2            # 6 head pairs
N_CORES = 8
GROUP = 4
EPS = 1e-5
SCALE = 1.0 / 8.0      # 1/sqrt(DK)

_CACHED = {}


def _layernorm(nc, pools, z, ln_sb, gcol, bcol, ones_f32, nm):
    """Post-LN over the feature (partition) dim of z: [6][128, Q] f32.
    Destroys z in place. Returns (x_f32, x_bf) lists [6][128,Q]."""
    sb, ps, small = pools["sb"], pools["ps"], pools["small"]
    m_ps = ps.tile([1, Q], F32, space="PSUM", name=f"mps_{nm}", tag="aux", bufs=2)
    s_ps = ps.tile([1, Q], F32, space="PSUM", name=f"ssps_{nm}", tag="aux", bufs=2)
    for dc in range(DC):
        sqt = sb.tile([P, Q], F32, name=f"sq_{nm}_{dc}", tag="lntmp", bufs=2)
        nc.scalar.activation(sqt[:], z[dc][:], AF.Square)
        nc.tensor.matmul(m_ps[:], ones_f32[:, 0:1], z[dc][:],
                         start=(dc == 0), stop=(dc == DC - 1))
        nc.tensor.matmul(s_ps[:], ones_f32[:, 0:1], sqt[:],
                         start=(dc == 0), stop=(dc == DC - 1))
    m = small.tile([1, Q], F32, name=f"m_{nm}", tag="m", bufs=1)
    nc.vector.tensor_scalar_mul(m[:], m_ps[:], 1.0 / D)
    msq = small.tile([1, Q], F32, name=f"msq_{nm}", tag="msq", bufs=1)
    nc.vector.tensor_scalar_mul(msq[:], s_ps[:], 1.0 / D)
    m2 = small.tile([1, Q], F32, name=f"m2_{nm}", tag="m2", bufs=1)
    nc.vector.tensor_tensor(m2[:], m[:], m[:], op=ALU.mult)
    # var -> msq (in place), std -> m2 (in place), rstd -> msq
    nc.vector.tensor_sub(msq[:], msq[:], m2[:])
    nc.scalar.activation(m2[:], msq[:], AF.Sqrt, bias=pools["eps"][0:1, 0:1])
    nc.vector.reciprocal(msq[:], m2[:])
    mb = sb.tile([P, Q], F32, name=f"mb_{nm}", tag="lntmp", bufs=2)
    nc.gpsimd.partition_broadcast(mb[:], m[:])
    rstdb = sb.tile([P, Q], F32, name=f"rstdb_{nm}", tag="lntmp", bufs=2)
    nc.gpsimd.partition_broadcast(rstdb[:], msq[:])
    xf, xb = [], []
    for dc in range(DC):
        nc.vector.tensor_sub(z[dc][:], z[dc][:], mb[:])
        nc.vector.tensor_tensor(z[dc][:], z[dc][:], rstdb[:], op=ALU.mult)
        xft = sb.tile([P, Q], F32, name=f"xf_{nm}_{dc}", tag=f"xout{dc}", bufs=2)
        nc.scalar.activation(xft[:], z[dc][:], AF.Identity,
                             scale=ln_sb[dc][:, gcol:gcol + 1],
                             bias=ln_sb[dc][:, bcol:bcol + 1])
        xbt = sb.tile([P, Q], BF16, name=f"xb_{nm}_{dc}", tag=f"xoutb{dc}", bufs=2)
        nc.vector.tensor_copy(xbt[:], xft[:])
        xf.append(xft)
        xb.append(xbt)
    return xf, xb


def _build_layer(nc, pools, dtens, l, xfull_bf, xq_bf, x_resid, ones_bf, ones_f32,
                 keep_sb, ln_sb):
    """Emit one decoder layer. x_resid is consumed in place.
    Returns (x_f32, x_bf) lists of [6][128,Q]."""
    sb, ps, small = pools["sb"], pools["ps"], pools["small"]
    wq_t, wk_t, wv_t, wo_t, w1_t, w2_t, b1_t, b2_t, attn_out = dtens

    b1_sb = sb.tile([P, FC], F32, name=f"b1sb{l}", tag="b1", bufs=2)
    nc.sync.dma_start(b1_sb[:],
                      b1_t.ap().rearrange("l (c p) -> p l c", p=P)[:, l, :])
    b2_sb = sb.tile([P, DC], F32, name=f"b2sb{l}", tag="b2", bufs=2)
    nc.sync.dma_start(b2_sb[:],
                      b2_t.ap().rearrange("l (c p) -> p l c", p=P)[:, l, :])

    def load_w(dram_t, nm):
        t = sb.tile([P, DC, D], BF16, name=nm, tag="wbig", bufs=1)
        nc.gpsimd.dma_start(t[:],
                            dram_t.ap()[l].rearrange("(dc p) f -> p dc f", p=P))
        return t

    # ---- q projection (head pair per psum tile: lhsT M=128 covers 2 heads) ----
    wq_sb = load_w(wq_t, f"wq{l}")
    qT2 = []
    for hc in range(HC):
        q_ps = ps.tile([P, Q], F32, space="PSUM", name=f"qps{l}_{hc}", tag="proj",
                       bufs=2)
        for dc in range(DC):
            nc.tensor.matmul(q_ps[:], wq_sb[:, dc, P * hc:P * (hc + 1)],
                             xq_bf[dc][:], start=(dc == 0), stop=(dc == DC - 1))
        qh = sb.tile([P, Q], BF16, name=f"qT{l}_{hc}", tag=f"qT{hc}", bufs=1)
        nc.scalar.copy(qh[:], q_ps[:])
        qT2.append(qh)

    # ---- k projection ----
    wk_sb = load_w(wk_t, f"wk{l}")
    kT2 = []
    for hc in range(HC):
        kh = sb.tile([P, S], BF16, name=f"kT{l}_{hc}", tag=f"kT{hc}", bufs=1)
        for t4 in range(S // 512):
            k_ps = ps.tile([P, 512], F32, space="PSUM", name=f"kps{l}_{hc}_{t4}",
                           tag="proj", bufs=2)
            for dc in range(DC):
                nc.tensor.matmul(k_ps[:], wk_sb[:, dc, P * hc:P * (hc + 1)],
                                 xfull_bf[dc][:, 512 * t4:512 * (t4 + 1)],
                                 start=(dc == 0), stop=(dc == DC - 1))
            nc.scalar.copy(kh[:, 512 * t4:512 * (t4 + 1)], k_ps[:])
        kT2.append(kh)

    # ---- v natural per token chunk: v_sb[c] [128, H, 64] bf16 ----
    wv_sb = load_w(wv_t, f"wv{l}")
    v_sb = []
    for c in range(TC):
        vt = sb.tile([P, H, DK], BF16, name=f"v{l}_{c}", tag=f"v{c}", bufs=1)
        for half in range(2):
            v_ps = ps.tile([P, 384], F32, space="PSUM", name=f"vps{l}_{c}_{half}",
                           tag="proj", bufs=2)
            for dc in range(DC):
                nc.tensor.matmul(v_ps[:], xfull_bf[dc][:, P * c:P * (c + 1)],
                                 wv_sb[:, dc, 384 * half:384 * (half + 1)],
                                 start=(dc == 0), stop=(dc == DC - 1))
            nc.vector.tensor_copy(vt[:, 6 * half:6 * (half + 1), :], v_ps[:])
        v_sb.append(vt)

    # ---- attention, head pairs, software-pipelined over tk chunks ----
    # prev = (l, h, um tiles, rb_bf) of the previous head unit whose
    # normalize+store is interleaved into the current head's chunk loop.
    prev = None

    def emit_scores(hc, sub, c):
        pr = slice(64 * sub, 64 * sub + 64)
        s_ps = ps.tile([P, Q], F32, space="PSUM",
                       name=f"sps{l}_{2 * hc + sub}_{c}", tag="scores", bufs=3)
        nc.tensor.matmul(s_ps[:], kT2[hc][pr, P * c:P * (c + 1)], qT2[hc][pr, :],
                         start=True, stop=True)
        return s_ps

    def emit_norm(pv, c):
        pl, ph, pum, prb = pv
        at = sb.tile([P, Q], BF16, name=f"at{pl}_{ph}_{c}", tag="attnstage",
                     bufs=2)
        nc.vector.tensor_tensor(at[:], pum[c][:], prb[:], op=ALU.mult)
        nc.sync.dma_start(attn_out.ap()[pl, ph, c], at[:])

    ctxT2 = []
    for hc in range(HC):
        ch = sb.tile([P, Q], BF16, name=f"ctxT{l}_{hc}", tag=f"ctxT{hc}", bufs=1)
        ctx_ps = ps.tile([P, Q], F32, space="PSUM", name=f"ctxps{l}_{hc}",
                         tag="ctx", bufs=1)
        for sub in range(2):
            h = 2 * hc + sub
            pr = slice(64 * sub, 64 * sub + 64)
            sum_ps = ps.tile([1, Q], F32, space="PSUM", name=f"sumps{l}_{h}",
                             tag="aux", bufs=2)
            um = []
            s_pipe = [emit_scores(hc, sub, 0), emit_scores(hc, sub, 1)]
            for c in range(TC):
                if c + 2 < TC:
                    s_pipe.append(emit_scores(hc, sub, c + 2))
                if prev is not None:
                    emit_norm(prev, c)
                s_ps = s_pipe[c]
                um_c = sb.tile([P, Q], BF16, name=f"um{l}_{h}_{c}", tag=f"um{c}",
                               bufs=1)
                nc.scalar.activation(um_c[:], s_ps[:], AF.Exp, scale=SCALE)
                nc.vector.tensor_tensor(um_c[:], um_c[:], keep_sb[c][:],
                                        op=ALU.mult)
                nc.tensor.matmul(ctx_ps[pr, :], v_sb[c][:, h, :], um_c[:],
                                 start=(c == 0), stop=(c == TC - 1),
                                 tile_position=(0, 64 * sub))
                nc.tensor.matmul(sum_ps[:], ones_bf[:, 0:1], um_c[:],
                                 start=(c == 0), stop=(c == TC - 1))
                um.append(um_c)
            # wide reciprocal: psum sums -> sbuf row -> broadcast -> recip(bf16)
            sums = small.tile([1, Q], F32, name=f"sums{l}_{h}", tag="recip",
                              bufs=2)
            nc.scalar.copy(sums[:], sum_ps[:])
            rbs = sb.tile([P, Q], F32, name=f"rbs{l}_{h}", tag="rbsrc", bufs=1)
            nc.gpsimd.partition_broadcast(rbs[:], sums[:])
            rb = sb.tile([P, Q], BF16, name=f"rb{l}_{h}", tag="rb", bufs=2)
            with nc.allow_low_precision(reason="softmax recip scale in bf16"):
                nc.vector.reciprocal(rb[:], rbs[:])
            nc.vector.tensor_tensor(ch[pr, :], ctx_ps[pr, :], rb[pr, :],
                                    op=ALU.mult)
            prev = (l, h, um, rb)
        ctxT2.append(ch)
    # flush the last head unit's normalize+store
    for c in range(TC):
        emit_norm(prev, c)

    # ---- wo + residual (in-place into x_resid) + LN1 ----
    wo_sb = load_w(wo_t, f"wo{l}")
    for dc in range(DC):
        o_ps = ps.tile([P, Q], F32, space="PSUM", name=f"ops{l}_{dc}", tag="proj",
                       bufs=2)
        for hc in range(HC):
            nc.tensor.matmul(o_ps[:], wo_sb[:, hc, P * dc:P * (dc + 1)],
                             ctxT2[hc][:], start=(hc == 0), stop=(hc == HC - 1))
        nc.vector.tensor_add(x_resid[dc][:], o_ps[:], x_resid[dc][:])
    x1f, x1b = _layernorm(nc, pools, x_resid, ln_sb, 4 * l + 0, 4 * l + 1,
                          ones_f32, f"ln1_{l}")

    # ---- FFN: single pass, 6 concurrent accumulators across psum tags ----
    f_tags = ["scores", "scores", "scores", "ctx", "aux", "aux"]
    f_bufs = {"scores": 3, "ctx": 1, "aux": 2}
    f_ps = [ps.tile([P, Q], F32, space="PSUM", name=f"fps{l}_{dc}",
                    tag=f_tags[dc], bufs=f_bufs[f_tags[dc]])
            for dc in range(DC)]
    for fc in range(FC):
        w1t = sb.tile([P, DC, P], BF16, name=f"w1_{l}_{fc}", tag="w1s", bufs=2)
        nc.gpsimd.dma_start(
            w1t[:],
            w1_t.ap()[l].rearrange("(dc p) f -> p dc f",
                                   p=P)[:, :, P * fc:P * (fc + 1)])
        h_ps = ps.tile([P, Q], F32, space="PSUM", name=f"hps{l}_{fc}",
                       tag="proj", bufs=2)
        for dc in range(DC):
            nc.tensor.matmul(h_ps[:], w1t[:, dc, :], x1b[dc][:],
                             start=(dc == 0), stop=(dc == DC - 1))
        ht = sb.tile([P, Q], BF16, name=f"hT{l}_{fc}", tag=f"um{fc % 4}", bufs=1)
        nc.scalar.activation(ht[:], h_ps[:], AF.Relu, bias=b1_sb[:, fc:fc + 1])
        w2t = sb.tile([P, DC, P], BF16, name=f"w2_{l}_{fc}", tag="w2s", bufs=3)
        nc.gpsimd.dma_start(
            w2t[:],
            w2_t.ap()[l].rearrange("(fc p) f -> p fc f", p=P)[:, fc, :]
            .rearrange("p (c q) -> p c q", q=P))
        for dc in range(DC):
            nc.tensor.matmul(f_ps[dc][:], w2t[:, dc, :], ht[:],
                             start=(fc == 0), stop=(fc == FC - 1))
    z2 = []
    for dc in range(DC):
        zt = sb.tile([P, Q], F32, name=f"z2_{l}_{dc}", tag=f"resid{dc}", bufs=1)
        nc.scalar.activation(zt[:], f_ps[dc][:], AF.Identity,
                             bias=b2_sb[:, dc:dc + 1])
        nc.vector.tensor_add(zt[:], zt[:], x1f[dc][:])
        z2.append(zt)
    return _layernorm(nc, pools, z2, ln_sb, 4 * l + 2, 4 * l + 3, ones_f32,
                      f"ln2_{l}")


def build_program():
    nc = bacc.Bacc("TRN2", target_bir_lowering=False, debug=False,
                   enable_asserts=False, num_devices=N_CORES)

    emb_t = nc.dram_tensor("emb", [VOCAB, D], F32, kind="ExternalInput")
    pos_full_t = nc.dram_tensor("pos_full", [S, D], F32, kind="ExternalInput")
    pos_loc_t = nc.dram_tensor("pos_loc", [Q, D], F32, kind="ExternalInput")
    idx_full_t = nc.dram_tensor("idx_full", [TC, P, 1], I32, kind="ExternalInput")
    idx_loc_t = nc.dram_tensor("idx_loc", [QC, P, 1], I32, kind="ExternalInput")
    keep_t = nc.dram_tensor("keep", [TC, P, Q], BF16, kind="ExternalInput")
    wq_t = nc.dram_tensor("wq", [L, D, D], F32, kind="ExternalInput")
    wk_t = nc.dram_tensor("wk", [L, D, D], F32, kind="ExternalInput")
    wv_t = nc.dram_tensor("wv", [L, D, D], F32, kind="ExternalInput")
    wo_t = nc.dram_tensor("wo", [L, D, D], F32, kind="ExternalInput")
    w1_t = nc.dram_tensor("w1", [L, D, DFF], F32, kind="ExternalInput")
    w2_t = nc.dram_tensor("w2", [L, DFF, D], F32, kind="ExternalInput")
    b1_t = nc.dram_tensor("b1", [L, DFF], F32, kind="ExternalInput")
    b2_t = nc.dram_tensor("b2", [L, D], F32, kind="ExternalInput")
    ln_t = nc.dram_tensor("lnp", [L * 4, D], F32, kind="ExternalInput")

    attn_out = nc.dram_tensor("attn_out", [L, H, TC, P, Q], BF16,
                              kind="ExternalOutput")
    x_out = nc.dram_tensor("x_out", [DC, P, Q], F32, kind="ExternalOutput")

    with tile.TileContext(nc, num_cores=N_CORES) as tc:
        with tc.tile_pool(name="sb", bufs=1) as sb, \
             tc.tile_pool(name="small", bufs=1) as small, \
             tc.tile_pool(name="ps", bufs=1, space="PSUM") as ps, \
             tc.tile_pool(name="dram", bufs=1, space="DRAM") as dram:
            pools = dict(sb=sb, ps=ps, small=small)

            ident = sb.tile([P, P], F32, name="ident", tag="ident", bufs=1)
            make_identity(nc, ident[:])
            ones_bf = sb.tile([P, 1], BF16, name="ones_bf", tag="ones_bf", bufs=1)
            nc.gpsimd.memset(ones_bf[:], 1.0)
            ones_f32 = sb.tile([P, 1], F32, name="ones_f32", tag="ones_f32",
                               bufs=1)
            nc.gpsimd.memset(ones_f32[:], 1.0)
            eps_sb = small.tile([1, 1], F32, name="eps_sb", tag="eps", bufs=1)
            nc.gpsimd.memset(eps_sb[:], EPS)
            pools["eps"] = eps_sb
            ln_sb = []
            for dc in range(DC):
                t = sb.tile([P, L * 4], F32, name=f"ln_sb{dc}", tag=f"ln{dc}",
                            bufs=1)
                nc.sync.dma_start(
                    t[:], ln_t.ap().rearrange("r (c p) -> p r c", p=P)[:, :, dc])
                ln_sb.append(t)
            keep_sb = []
            for c in range(TC):
                t = sb.tile([P, Q], BF16, name=f"keep{c}", tag=f"keep{c}", bufs=1)
                nc.sync.dma_start(t[:], keep_t.ap()[c])
                keep_sb.append(t)

            x1_bf = [sb.tile([P, S], BF16, name=f"x1bf{dc}", tag=f"xfull{dc}",
                             bufs=1) for dc in range(DC)]
            x_resid = [sb.tile([P, Q], F32, name=f"xres{dc}", tag=f"resid{dc}",
                               bufs=1) for dc in range(DC)]
            xq_bf = [sb.tile([P, Q], BF16, name=f"xqbf{dc}", tag=f"xoutb{dc}",
                             bufs=2) for dc in range(DC)]

            def gather_chunk(idx_ap, pos_ap, c, dsts):
                idx_sb = sb.tile([P, 1], I32, name=f"idx{c}", tag="idx", bufs=2)
                nc.sync.dma_start(idx_sb[:], idx_ap)
                xn = sb.tile([P, D], F32, name=f"xn{c}", tag="nat", bufs=2)
                nc.gpsimd.indirect_dma_start(
                    out=xn[:], out_offset=None, in_=emb_t.ap(),
                    in_offset=bass.IndirectOffsetOnAxis(ap=idx_sb[:, 0:1], axis=0))
                pn = sb.tile([P, D], F32, name=f"pn{c}", tag="nat", bufs=2)
                nc.sync.dma_start(pn[:], pos_ap)
                nc.vector.tensor_add(xn[:], xn[:], pn[:])
                for dc in range(DC):
                    t_ps = ps.tile([P, P], F32, space="PSUM", name=f"tp{c}_{dc}",
                                   tag="aux", bufs=2)
                    nc.tensor.transpose(t_ps[:], xn[:, P * dc:P * (dc + 1)],
                                        ident[:])
                    dsts(dc, t_ps)

            for c in range(TC):
                def dst_full(dc, t_ps, c=c):
                    nc.scalar.copy(x1_bf[dc][:, P * c:P * (c + 1)], t_ps[:])
                gather_chunk(idx_full_t.ap()[c],
                             pos_full_t.ap()[P * c:P * (c + 1), :], c, dst_full)
            for c in range(QC):
                def dst_loc(dc, t_ps, c=c):
                    nc.vector.tensor_copy(x_resid[dc][:, P * c:P * (c + 1)],
                                          t_ps[:])
                    nc.scalar.copy(xq_bf[dc][:, P * c:P * (c + 1)], t_ps[:])
                gather_chunk(idx_loc_t.ap()[c],
                             pos_loc_t.ap()[P * c:P * (c + 1), :], TC + c,
                             dst_loc)

            dtens = (wq_t, wk_t, wv_t, wo_t, w1_t, w2_t, b1_t, b2_t, attn_out)

            x2f, x2b = _build_layer(nc, pools, dtens, 0, x1_bf, xq_bf, x_resid,
                                    ones_bf, ones_f32, keep_sb, ln_sb)

            cc_in = dram.tile([DC, P, Q], BF16, name="cc_in", tag="cc_in")
            for dc in range(DC):
                nc.sync.dma_start(cc_in[dc], x2b[dc][:])
            cc_out = dram.tile([GROUP, DC, P, Q], BF16, name="cc_out",
                               tag="cc_out")
            nc.gpsimd.collective_compute(
                "AllGather", ALU.bypass,
                replica_groups=[[0, 1, 2, 3], [4, 5, 6, 7]],
                ins=[cc_in[:]], outs=[cc_out[:]])
            x2_full = [sb.tile([P, S], BF16, name=f"x2bf{dc}", tag=f"xfull{dc}",
                               bufs=1) for dc in range(DC)]
            for g in range(GROUP):
                for dc in range(DC):
                    nc.sync.dma_start(x2_full[dc][:, Q * g:Q * (g + 1)],
                                      cc_out[g, dc])

            x3f, _x3b = _build_layer(nc, pools, dtens, 1, x2_full, x2b, x2f,
                                     ones_bf, ones_f32, keep_sb, ln_sb)

            for dc in range(DC):
                nc.sync.dma_start(x_out.ap()[dc], x3f[dc][:])

    nc.compile()
    return nc


def _get_program():
    if "nc" not in _CACHED:
        _CACHED["nc"] = build_program()
    return _CACHED["nc"]


def _prep_in_maps(inputs):
    idx = np.asarray(inputs["inputs"]).astype(np.int32)            # [B, S]
    amask = np.asarray(inputs["attention_mask"]).astype(np.int32)  # [B, S]
    pos = np.ascontiguousarray(np.asarray(inputs["pos_emb"], np.float32)[:S])
    lnp = np.stack([inputs["ln1_g"], inputs["ln1_b"],
                    inputs["ln2_g"], inputs["ln2_b"]], axis=1)     # [L, 4, D]
    lnp = np.ascontiguousarray(lnp.reshape(L * 4, D).astype(np.float32))

    shared = {
        "emb": np.ascontiguousarray(np.asarray(inputs["emb"], np.float32)),
        "pos_full": pos,
        "wq": np.ascontiguousarray(np.asarray(inputs["wq"], np.float32)),
        "wk": np.ascontiguousarray(np.asarray(inputs["wk"], np.float32)),
        "wv": np.ascontiguousarray(np.asarray(inputs["wv"], np.float32)),
        "wo": np.ascontiguousarray(np.asarray(inputs["wo"], np.float32)),
        "w1": np.ascontiguousarray(np.asarray(inputs["w1"], np.float32)),
        "w2": np.ascontiguousarray(np.asarray(inputs["w2"], np.float32)),
        "b1": np.ascontiguousarray(np.asarray(inputs["b1"], np.float32)),
        "b2": np.ascontiguousarray(np.asarray(inputs["b2"], np.float32)),
        "lnp": lnp,
    }
    tk = np.arange(S)[:, None]                     # [S, 1] key positions
    in_maps = []
    for core in range(N_CORES):
        b, r = divmod(core, GROUP)
        o = Q * r
        tq = np.arange(o, o + Q)[None, :]          # [1, Q] query positions
        pad = (amask[b] == 0)[:, None]             # [S, 1]
        keep = (pad | (tk > tq)).astype(np.float32)  # [S, Q]
        m = dict(shared)
        m["pos_loc"] = np.ascontiguousarray(pos[o:o + Q])
        m["idx_full"] = np.ascontiguousarray(idx[b].reshape(TC, P, 1))
        m["idx_loc"] = np.ascontiguousarray(idx[b, o:o + Q].reshape(QC, P, 1))
        m["keep"] = np.ascontiguousarray(
            keep.reshape(TC, P, Q).astype(ml_dtypes.bfloat16))
        in_maps.append(m)
    return in_maps


def kernel(**inputs):
    nc = _get_program()
    in_maps = _prep_in_maps(inputs)
    res = run_bass_kernel_spmd(nc, in_maps, core_ids=list(range(N_CORES)))
    x = np.zeros((B, S, D), np.float32)
    attns = np.zeros((L, B, H, S, S), np.float32)
    for core in range(N_CORES):
        b, r = divmod(core, GROUP)
        o = Q * r
        out = res.results[core]
        x[b, o:o + Q, :] = out["x_out"].reshape(D, Q).T
        a = out["attn_out"].reshape(L, H, S, Q).astype(np.float32)
        attns[:, b, :, o:o + Q, :] = a.transpose(0, 1, 3, 2)
    return x, attns


if __name__ == "__main__":
    _get_program()
    print("program built ok")


# revision 18
# speedup vs baseline: 1.1452x; 1.0594x over previous
"""Trainium2 Bass kernel for a 2-layer post-LN decoder (nn_Decoder_54082228191615).

Sharding: 8 cores = 2 batch groups x 4 query-slice ranks.
Core (b, r) computes all 12 heads for query tokens [512r, 512r+512) of batch b,
writes its attn-prob slice transposed ([tk, tq_local], bf16), and its slice of
the residual stream. One bf16 AllGather per batch group provides the full
layer-1 output for layer-2 K/V. Host reassembles/transposes the full outputs.

All on-chip activations are kept transposed [feature, token] so matmuls need
no on-chip transposes; softmax denominators come from ones-vector matmuls.
Heads are packed in pairs onto the 128-wide partition dim (tile_position).
"""

import numpy as np
import ml_dtypes

import concourse.bass as bass
import concourse.tile as tile
from concourse import bacc, mybir
from concourse.bass_utils import run_bass_kernel_spmd
from concourse.masks import make_identity

F32 = mybir.dt.float32
BF16 = mybir.dt.bfloat16
I32 = mybir.dt.int32
AF = mybir.ActivationFunctionType
ALU = mybir.AluOpType

B, S, D, H, DK, DFF, L = 2, 2048, 768, 12, 64, 3072, 2
VOCAB = 32000
P = 128
DC = D // P            # 6 feature chunks
TC = S // P            # 16 token chunks
Q = 512                # local query-slice width
QC = Q // P            # 4 local token chunks
FC = DFF // P          # 24 ffn chunks
HC = H // 2            # 6 head pairs
N_CORES = 8
GROUP = 4
EPS = 1e-5
SCALE = 1.0 / 8.0      # 1/sqrt(DK)

_CACHED = {}


def _layernorm(nc, pools, z, ln_sb, gcol, bcol, ones_f32, nm):
    """Post-LN over the feature (partition) dim of z: [6][128, Q] f32.
    Destroys z in place. Returns (x_f32, x_bf) lists [6][128,Q]."""
    sb, ps, small = pools["sb"], pools["ps"], pools["small"]
    m_ps = ps.tile([1, Q], F32, space="PSUM", name=f"mps_{nm}", tag="aux", bufs=2)
    s_ps = ps.tile([1, Q], F32, space="PSUM", name=f"ssps_{nm}", tag="aux", bufs=2)
    for dc in range(DC):
        sqt = sb.tile([P, Q], F32, name=f"sq_{nm}_{dc}", tag="lntmp", bufs=2)
        nc.scalar.activation(sqt[:], z[dc][:], AF.Square)
        nc.tensor.matmul(m_ps[:], ones_f32[:, 0:1], z[dc][:],
                         start=(dc == 0), stop=(dc == DC - 1))
        nc.tensor.matmul(s_ps[:], ones_f32[:, 0:1], sqt[:],
                         start=(dc == 0), stop=(dc == DC - 1))
    m = small.tile([1, Q], F32, name=f"m_{nm}", tag="m", bufs=1)
    nc.vector.tensor_scalar_mul(m[:], m_ps[:], 1.0 / D)
    msq = small.tile([1, Q], F32, name=f"msq_{nm}", tag="msq", bufs=1)
    nc.vector.tensor_scalar_mul(msq[:], s_ps[:], 1.0 / D)
    m2 = small.tile([1, Q], F32, name=f"m2_{nm}", tag="m2", bufs=1)
    nc.vector.tensor_tensor(m2[:], m[:], m[:], op=ALU.mult)
    # var -> msq (in place), std -> m2 (in place)
    nc.vector.tensor_sub(msq[:], msq[:], m2[:])
    nc.scalar.activation(m2[:], msq[:], AF.Sqrt, bias=pools["eps"][0:1, 0:1])
    # broadcast mean and std across partitions on PE, then wide fast recip
    ones_row = pools["ones_row"]
    mb_ps = ps.tile([P, Q], F32, space="PSUM", name=f"mbps_{nm}", tag="aux",
                    bufs=2)
    nc.tensor.matmul(mb_ps[:], ones_row[0:1, :], m[:], start=True, stop=True)
    stdb_ps = ps.tile([P, Q], F32, space="PSUM", name=f"stdbps_{nm}", tag="aux",
                     bufs=2)
    nc.tensor.matmul(stdb_ps[:], ones_row[0:1, :], m2[:], start=True, stop=True)
    rstdb = sb.tile([P, Q], F32, name=f"rstdb_{nm}", tag="lntmp", bufs=2)
    nc.vector.reciprocal_approx_fast(rstdb[:], stdb_ps[:])
    xf, xb = [], []
    for dc in range(DC):
        nc.vector.tensor_sub(z[dc][:], z[dc][:], mb_ps[:])
        nc.vector.tensor_tensor(z[dc][:], z[dc][:], rstdb[:], op=ALU.mult)
        xft = sb.tile([P, Q], F32, name=f"xf_{nm}_{dc}", tag=f"xout{dc}", bufs=2)
        nc.scalar.activation(xft[:], z[dc][:], AF.Identity,
                             scale=ln_sb[dc][:, gcol:gcol + 1],
                             bias=ln_sb[dc][:, bcol:bcol + 1])
        xbt = sb.tile([P, Q], BF16, name=f"xb_{nm}_{dc}", tag=f"xoutb{dc}", bufs=2)
        nc.vector.tensor_copy(xbt[:], xft[:])
        xf.append(xft)
        xb.append(xbt)
    return xf, xb


def _build_layer(nc, pools, dtens, l, xfull_bf, xq_bf, x_resid, ones_bf, ones_f32,
                 keep_sb, ln_sb):
    """Emit one decoder layer. x_resid is consumed in place.
    Returns (x_f32, x_bf) lists of [6][128,Q]."""
    sb, ps, small = pools["sb"], pools["ps"], pools["small"]
    wq_t, wk_t, wv_t, wo_t, w1_t, w2_t, b1_t, b2_t, attn_out = dtens

    b1_sb = sb.tile([P, FC], F32, name=f"b1sb{l}", tag="b1", bufs=2)
    nc.sync.dma_start(b1_sb[:],
                      b1_t.ap().rearrange("l (c p) -> p l c", p=P)[:, l, :])
    b2_sb = sb.tile([P, DC], F32, name=f"b2sb{l}", tag="b2", bufs=2)
    nc.sync.dma_start(b2_sb[:],
                      b2_t.ap().rearrange("l (c p) -> p l c", p=P)[:, l, :])

    def load_w(dram_t, nm):
        t = sb.tile([P, DC, D], BF16, name=nm, tag="wbig", bufs=1)
        nc.gpsimd.dma_start(t[:],
                            dram_t.ap()[l].rearrange("(dc p) f -> p dc f", p=P))
        return t

    # ---- q projection (head pair per psum tile: lhsT M=128 covers 2 heads) ----
    wq_sb = load_w(wq_t, f"wq{l}")
    qT2 = []
    for hc in range(HC):
        q_ps = ps.tile([P, Q], F32, space="PSUM", name=f"qps{l}_{hc}", tag="proj",
                       bufs=2)
        for dc in range(DC):
            nc.tensor.matmul(q_ps[:], wq_sb[:, dc, P * hc:P * (hc + 1)],
                             xq_bf[dc][:], start=(dc == 0), stop=(dc == DC - 1))
        qh = sb.tile([P, Q], BF16, name=f"qT{l}_{hc}", tag=f"qT{hc}", bufs=1)
        nc.scalar.copy(qh[:], q_ps[:])
        qT2.append(qh)

    # ---- v natural per token chunk: v_sb[c] [128, H, 64] bf16 ----
    wv_sb = load_w(wv_t, f"wv{l}")
    v_sb = []
    for c in range(TC):
        vt = sb.tile([P, H, DK], BF16, name=f"v{l}_{c}", tag=f"v{c}", bufs=1)
        for half in range(2):
            v_ps = ps.tile([P, 384], F32, space="PSUM", name=f"vps{l}_{c}_{half}",
                           tag="proj", bufs=2)
            for dc in range(DC):
                nc.tensor.matmul(v_ps[:], xfull_bf[dc][:, P * c:P * (c + 1)],
                                 wv_sb[:, dc, 384 * half:384 * (half + 1)],
                                 start=(dc == 0), stop=(dc == DC - 1))
            nc.vector.tensor_copy(vt[:, 6 * half:6 * (half + 1), :], v_ps[:])
        v_sb.append(vt)

    # ---- k projection (pair 0 upfront; pairs 1..5 interleaved as PE filler
    # into the previous pair's attention loop) ----
    wk_sb = load_w(wk_t, f"wk{l}")
    kT2 = [sb.tile([P, S], BF16, name=f"kT{l}_{hc}", tag=f"kT{hc}", bufs=1)
           for hc in range(HC)]
    kfill_state = {}

    def emit_k_step(hc, j):
        """Filler unit j (0..23) of pair hc's k projection: one matmul,
        plus the psum drain after each group of DC."""
        t4, dc = divmod(j, DC)
        if dc == 0:
            kfill_state[hc] = ps.tile([P, 512], F32, space="PSUM",
                                      name=f"kps{l}_{hc}_{t4}", tag="proj",
                                      bufs=2)
        k_ps = kfill_state[hc]
        nc.tensor.matmul(k_ps[:], wk_sb[:, dc, P * hc:P * (hc + 1)],
                         xfull_bf[dc][:, 512 * t4:512 * (t4 + 1)],
                         start=(dc == 0), stop=(dc == DC - 1))
        if dc == DC - 1:
            nc.scalar.copy(kT2[hc][:, 512 * t4:512 * (t4 + 1)], k_ps[:])

    for j in range(4 * DC):
        emit_k_step(0, j)

    # ---- attention, head pairs, software-pipelined over tk chunks ----
    # prev = (l, h, um tiles, rb_bf) of the previous head unit whose
    # normalize+store is interleaved into the current head's chunk loop.
    prev = None

    def emit_scores(hc, sub, c):
        pr = slice(64 * sub, 64 * sub + 64)
        s_ps = ps.tile([P, Q], F32, space="PSUM",
                       name=f"sps{l}_{2 * hc + sub}_{c}", tag="scores", bufs=3)
        nc.tensor.matmul(s_ps[:], kT2[hc][pr, P * c:P * (c + 1)], qT2[hc][pr, :],
                         start=True, stop=True)
        return s_ps

    def emit_norm(pv, c):
        pl, ph, pum, prb = pv
        at = sb.tile([P, Q], BF16, name=f"at{pl}_{ph}_{c}", tag="attnstage",
                     bufs=2)
        nc.vector.tensor_tensor(at[:], pum[c][:], prb[:], op=ALU.mult)
        nc.sync.dma_start(attn_out.ap()[pl, ph, c], at[:])

    ones_row = pools["ones_row"]
    ctxT2 = []
    for hc in range(HC):
        ch = sb.tile([P, Q], BF16, name=f"ctxT{l}_{hc}", tag=f"ctxT{hc}", bufs=1)
        ctx_ps = ps.tile([P, Q], F32, space="PSUM", name=f"ctxps{l}_{hc}",
                         tag="ctx", bufs=1)
        for sub in range(2):
            h = 2 * hc + sub
            pr = slice(64 * sub, 64 * sub + 64)
            # k-projection filler for the next pair: 24 units over 2x16 iters
            fill = ([(hc + 1, j) for j in range(4 * DC)]
                    if (hc + 1 < HC and sub == 0) else [])
            sum_ps = ps.tile([1, Q], F32, space="PSUM", name=f"sumps{l}_{h}",
                             tag="aux", bufs=2)
            um = []
            s_pipe = [emit_scores(hc, sub, 0), emit_scores(hc, sub, 1)]
            for c in range(TC):
                if fill and c % 2 == 0:
                    emit_k_step(*fill.pop(0))
                if c + 2 < TC:
                    s_pipe.append(emit_scores(hc, sub, c + 2))
                if fill and c % 2 == 1:
                    emit_k_step(*fill.pop(0))
                if prev is not None:
                    emit_norm(prev, c)
                s_ps = s_pipe[c]
                um_c = sb.tile([P, Q], BF16, name=f"um{l}_{h}_{c}", tag=f"um{c}",
                               bufs=1)
                nc.scalar.activation(um_c[:], s_ps[:], AF.Exp, scale=SCALE)
                nc.vector.tensor_tensor(um_c[:], um_c[:], keep_sb[c][:],
                                        op=ALU.mult)
                nc.tensor.matmul(ctx_ps[pr, :], v_sb[c][:, h, :], um_c[:],
                                 start=(c == 0), stop=(c == TC - 1),
                                 tile_position=(0, 64 * sub))
                nc.tensor.matmul(sum_ps[:], ones_bf[:, 0:1], um_c[:],
                                 start=(c == 0), stop=(c == TC - 1))
                um.append(um_c)
            while fill:
                emit_k_step(*fill.pop(0))
            # sums -> sbuf row -> PE broadcast -> wide fast recip -> bf16
            sums = small.tile([1, Q], F32, name=f"sums{l}_{h}", tag="recip",
                              bufs=1)
            nc.scalar.copy(sums[:], sum_ps[:])
            rbs_ps = ps.tile([P, Q], F32, space="PSUM", name=f"rbsps{l}_{h}",
                             tag="proj", bufs=2)
            nc.tensor.matmul(rbs_ps[:], ones_row[0:1, :], sums[:],
                             start=True, stop=True)
            rbf = sb.tile([P, Q], F32, name=f"rbf{l}_{h}", tag="rbsrc", bufs=1)
            nc.vector.reciprocal_approx_fast(rbf[:], rbs_ps[:])
            rb = sb.tile([P, Q], BF16, name=f"rb{l}_{h}", tag="rb", bufs=2)
            nc.vector.tensor_copy(rb[:], rbf[:])
            nc.vector.tensor_tensor(ch[pr, :], ctx_ps[pr, :], rb[pr, :],
                                    op=ALU.mult)
            prev = (l, h, um, rb)
        ctxT2.append(ch)
    # flush the last head unit's normalize+store
    for c in range(TC):
        emit_norm(prev, c)

    # ---- wo + residual (in-place into x_resid) + LN1 ----
    wo_sb = load_w(wo_t, f"wo{l}")
    for dc in range(DC):
        o_ps = ps.tile([P, Q], F32, space="PSUM", name=f"ops{l}_{dc}", tag="proj",
                       bufs=2)
        for hc in range(HC):
            nc.tensor.matmul(o_ps[:], wo_sb[:, hc, P * dc:P * (dc + 1)],
                             ctxT2[hc][:], start=(hc == 0), stop=(hc == HC - 1))
        nc.vector.tensor_add(x_resid[dc][:], o_ps[:], x_resid[dc][:])
    x1f, x1b = _layernorm(nc, pools, x_resid, ln_sb, 4 * l + 0, 4 * l + 1,
                          ones_f32, f"ln1_{l}")

    # ---- FFN: single pass, 6 concurrent accumulators across psum tags ----
    f_tags = ["scores", "scores", "scores", "ctx", "aux", "aux"]
    f_bufs = {"scores": 3, "ctx": 1, "aux": 2}
    f_ps = [ps.tile([P, Q], F32, space="PSUM", name=f"fps{l}_{dc}",
                    tag=f_tags[dc], bufs=f_bufs[f_tags[dc]])
            for dc in range(DC)]
    for fc in range(FC):
        w1t = sb.tile([P, DC, P], BF16, name=f"w1_{l}_{fc}", tag="w1s", bufs=2)
        nc.gpsimd.dma_start(
            w1t[:],
            w1_t.ap()[l].rearrange("(dc p) f -> p dc f",
                                   p=P)[:, :, P * fc:P * (fc + 1)])
        h_ps = ps.tile([P, Q], F32, space="PSUM", name=f"hps{l}_{fc}",
                       tag="proj", bufs=2)
        for dc in range(DC):
            nc.tensor.matmul(h_ps[:], w1t[:, dc, :], x1b[dc][:],
                             start=(dc == 0), stop=(dc == DC - 1))
        ht = sb.tile([P, Q], BF16, name=f"hT{l}_{fc}", tag=f"um{fc % 4}", bufs=1)
        nc.scalar.activation(ht[:], h_ps[:], AF.Relu, bias=b1_sb[:, fc:fc + 1])
        w2t = sb.tile([P, DC, P], BF16, name=f"w2_{l}_{fc}", tag="w2s", bufs=3)
        nc.gpsimd.dma_start(
            w2t[:],
            w2_t.ap()[l].rearrange("(fc p) f -> p fc f", p=P)[:, fc, :]
            .rearrange("p (c q) -> p c q", q=P))
        for dc in range(DC):
            nc.tensor.matmul(f_ps[dc][:], w2t[:, dc, :], ht[:],
                             start=(fc == 0), stop=(fc == FC - 1))
    z2 = []
    for dc in range(DC):
        zt = sb.tile([P, Q], F32, name=f"z2_{l}_{dc}", tag=f"resid{dc}", bufs=1)
        nc.scalar.activation(zt[:], f_ps[dc][:], AF.Identity,
                             bias=b2_sb[:, dc:dc + 1])
        nc.vector.tensor_add(zt[:], zt[:], x1f[dc][:])
        z2.append(zt)
    return _layernorm(nc, pools, z2, ln_sb, 4 * l + 2, 4 * l + 3, ones_f32,
                      f"ln2_{l}")


def build_program():
    nc = bacc.Bacc("TRN2", target_bir_lowering=False, debug=False,
                   enable_asserts=False, num_devices=N_CORES)

    emb_t = nc.dram_tensor("emb", [VOCAB, D], F32, kind="ExternalInput")
    pos_full_t = nc.dram_tensor("pos_full", [S, D], F32, kind="ExternalInput")
    pos_loc_t = nc.dram_tensor("pos_loc", [Q, D], F32, kind="ExternalInput")
    idx_full_t = nc.dram_tensor("idx_full", [TC, P, 1], I32, kind="ExternalInput")
    idx_loc_t = nc.dram_tensor("idx_loc", [QC, P, 1], I32, kind="ExternalInput")
    keep_t = nc.dram_tensor("keep", [TC, P, Q], BF16, kind="ExternalInput")
    wq_t = nc.dram_tensor("wq", [L, D, D], F32, kind="ExternalInput")
    wk_t = nc.dram_tensor("wk", [L, D, D], F32, kind="ExternalInput")
    wv_t = nc.dram_tensor("wv", [L, D, D], F32, kind="ExternalInput")
    wo_t = nc.dram_tensor("wo", [L, D, D], F32, kind="ExternalInput")
    w1_t = nc.dram_tensor("w1", [L, D, DFF], F32, kind="ExternalInput")
    w2_t = nc.dram_tensor("w2", [L, DFF, D], F32, kind="ExternalInput")
    b1_t = nc.dram_tensor("b1", [L, DFF], F32, kind="ExternalInput")
    b2_t = nc.dram_tensor("b2", [L, D], F32, kind="ExternalInput")
    ln_t = nc.dram_tensor("lnp", [L * 4, D], F32, kind="ExternalInput")

    attn_out = nc.dram_tensor("attn_out", [L, H, TC, P, Q], BF16,
                              kind="ExternalOutput")
    x_out = nc.dram_tensor("x_out", [DC, P, Q], F32, kind="ExternalOutput")

    with tile.TileContext(nc, num_cores=N_CORES) as tc:
        with tc.tile_pool(name="sb", bufs=1) as sb, \
             tc.tile_pool(name="small", bufs=1) as small, \
             tc.tile_pool(name="ps", bufs=1, space="PSUM") as ps, \
             tc.tile_pool(name="dram", bufs=1, space="DRAM") as dram:
            pools = dict(sb=sb, ps=ps, small=small)

            ident = sb.tile([P, P], F32, name="ident", tag="ident", bufs=1)
            make_identity(nc, ident[:])
            ones_bf = sb.tile([P, 1], BF16, name="ones_bf", tag="ones_bf", bufs=1)
            nc.gpsimd.memset(ones_bf[:], 1.0)
            ones_f32 = sb.tile([P, 1], F32, name="ones_f32", tag="ones_f32",
                               bufs=1)
            nc.gpsimd.memset(ones_f32[:], 1.0)
            eps_sb = small.tile([1, 1], F32, name="eps_sb", tag="eps", bufs=1)
            nc.gpsimd.memset(eps_sb[:], EPS)
            pools["eps"] = eps_sb
            ones_row = small.tile([1, P], F32, name="ones_row", tag="ones_row",
                                  bufs=1)
            nc.gpsimd.memset(ones_row[:], 1.0)
            pools["ones_row"] = ones_row
            ln_sb = []
            for dc in range(DC):
                t = sb.tile([P, L * 4], F32, name=f"ln_sb{dc}", tag=f"ln{dc}",
                            bufs=1)
                nc.sync.dma_start(
                    t[:], ln_t.ap().rearrange("r (c p) -> p r c", p=P)[:, :, dc])
                ln_sb.append(t)
            keep_sb = []
            for c in range(TC):
                t = sb.tile([P, Q], BF16, name=f"keep{c}", tag=f"keep{c}", bufs=1)
                nc.sync.dma_start(t[:], keep_t.ap()[c])
                keep_sb.append(t)

            x1_bf = [sb.tile([P, S], BF16, name=f"x1bf{dc}", tag=f"xfull{dc}",
                             bufs=1) for dc in range(DC)]
            x_resid = [sb.tile([P, Q], F32, name=f"xres{dc}", tag=f"resid{dc}",
                               bufs=1) for dc in range(DC)]
            xq_bf = [sb.tile([P, Q], BF16, name=f"xqbf{dc}", tag=f"xoutb{dc}",
                             bufs=2) for dc in range(DC)]

            def gather_chunk(idx_ap, pos_ap, c, dsts):
                idx_sb = sb.tile([P, 1], I32, name=f"idx{c}", tag="idx", bufs=2)
                nc.sync.dma_start(idx_sb[:], idx_ap)
                xn = sb.tile([P, D], F32, name=f"xn{c}", tag="nat", bufs=2)
                nc.gpsimd.indirect_dma_start(
                    out=xn[:], out_offset=None, in_=emb_t.ap(),
                    in_offset=bass.IndirectOffsetOnAxis(ap=idx_sb[:, 0:1], axis=0))
                pn = sb.tile([P, D], F32, name=f"pn{c}", tag="nat", bufs=2)
                nc.sync.dma_start(pn[:], pos_ap)
                nc.vector.tensor_add(xn[:], xn[:], pn[:])
                for dc in range(DC):
                    t_ps = ps.tile([P, P], F32, space="PSUM", name=f"tp{c}_{dc}",
                                   tag="aux", bufs=2)
                    nc.tensor.transpose(t_ps[:], xn[:, P * dc:P * (dc + 1)],
                                        ident[:])
                    dsts(dc, t_ps)

            for c in range(TC):
                def dst_full(dc, t_ps, c=c):
                    nc.scalar.copy(x1_bf[dc][:, P * c:P * (c + 1)], t_ps[:])
                gather_chunk(idx_full_t.ap()[c],
                             pos_full_t.ap()[P * c:P * (c + 1), :], c, dst_full)
            for c in range(QC):
                def dst_loc(dc, t_ps, c=c):
                    nc.vector.tensor_copy(x_resid[dc][:, P * c:P * (c + 1)],
                                          t_ps[:])
                    nc.scalar.copy(xq_bf[dc][:, P * c:P * (c + 1)], t_ps[:])
                gather_chunk(idx_loc_t.ap()[c],
                             pos_loc_t.ap()[P * c:P * (c + 1), :], TC + c,
                             dst_loc)

            dtens = (wq_t, wk_t, wv_t, wo_t, w1_t, w2_t, b1_t, b2_t, attn_out)

            x2f, x2b = _build_layer(nc, pools, dtens, 0, x1_bf, xq_bf, x_resid,
                                    ones_bf, ones_f32, keep_sb, ln_sb)

            cc_in = dram.tile([DC, P, Q], BF16, name="cc_in", tag="cc_in")
            for dc in range(DC):
                nc.sync.dma_start(cc_in[dc], x2b[dc][:])
            cc_out = dram.tile([GROUP, DC, P, Q], BF16, name="cc_out",
                               tag="cc_out")
            nc.gpsimd.collective_compute(
                "AllGather", ALU.bypass,
                replica_groups=[[0, 1, 2, 3], [4, 5, 6, 7]],
                ins=[cc_in[:]], outs=[cc_out[:]])
            x2_full = [sb.tile([P, S], BF16, name=f"x2bf{dc}", tag=f"xfull{dc}",
                               bufs=1) for dc in range(DC)]
            for g in range(GROUP):
                for dc in range(DC):
                    nc.sync.dma_start(x2_full[dc][:, Q * g:Q * (g + 1)],
                                      cc_out[g, dc])

            x3f, _x3b = _build_layer(nc, pools, dtens, 1, x2_full, x2b, x2f,
                                     ones_bf, ones_f32, keep_sb, ln_sb)

            for dc in range(DC):
                nc.sync.dma_start(x_out.ap()[dc], x3f[dc][:])

    nc.compile()
    return nc


def _get_program():
    if "nc" not in _CACHED:
        _CACHED["nc"] = build_program()
    return _CACHED["nc"]


def _prep_in_maps(inputs):
    idx = np.asarray(inputs["inputs"]).astype(np.int32)            # [B, S]
    amask = np.asarray(inputs["attention_mask"]).astype(np.int32)  # [B, S]
    pos = np.ascontiguousarray(np.asarray(inputs["pos_emb"], np.float32)[:S])
    lnp = np.stack([inputs["ln1_g"], inputs["ln1_b"],
                    inputs["ln2_g"], inputs["ln2_b"]], axis=1)     # [L, 4, D]
    lnp = np.ascontiguousarray(lnp.reshape(L * 4, D).astype(np.float32))

    shared = {
        "emb": np.ascontiguousarray(np.asarray(inputs["emb"], np.float32)),
        "pos_full": pos,
        "wq": np.ascontiguousarray(np.asarray(inputs["wq"], np.float32)),
        "wk": np.ascontiguousarray(np.asarray(inputs["wk"], np.float32)),
        "wv": np.ascontiguousarray(np.asarray(inputs["wv"], np.float32)),
        "wo": np.ascontiguousarray(np.asarray(inputs["wo"], np.float32)),
        "w1": np.ascontiguousarray(np.asarray(inputs["w1"], np.float32)),
        "w2": np.ascontiguousarray(np.asarray(inputs["w2"], np.float32)),
        "b1": np.ascontiguousarray(np.asarray(inputs["b1"], np.float32)),
        "b2": np.ascontiguousarray(np.asarray(inputs["b2"], np.float32)),
        "lnp": lnp,
    }
    tk = np.arange(S)[:, None]                     # [S, 1] key positions
    in_maps = []
    for core in range(N_CORES):
        b, r = divmod(core, GROUP)
        o = Q * r
        tq = np.arange(o, o + Q)[None, :]          # [1, Q] query positions
        pad = (amask[b] == 0)[:, None]             # [S, 1]
        keep = (pad | (tk > tq)).astype(np.float32)  # [S, Q]
        m = dict(shared)
        m["pos_loc"] = np.ascontiguousarray(pos[o:o + Q])
        m["idx_full"] = np.ascontiguousarray(idx[b].reshape(TC, P, 1))
        m["idx_loc"] = np.ascontiguousarray(idx[b, o:o + Q].reshape(QC, P, 1))
        m["keep"] = np.ascontiguousarray(
            keep.reshape(TC, P, Q).astype(ml_dtypes.bfloat16))
        in_maps.append(m)
    return in_maps


def kernel(**inputs):
    nc = _get_program()
    in_maps = _prep_in_maps(inputs)
    res = run_bass_kernel_spmd(nc, in_maps, core_ids=list(range(N_CORES)))
    x = np.zeros((B, S, D), np.float32)
    attns = np.zeros((L, B, H, S, S), np.float32)
    for core in range(N_CORES):
        b, r = divmod(core, GROUP)
        o = Q * r
        out = res.results[core]
        x[b, o:o + Q, :] = out["x_out"].reshape(D, Q).T
        a = out["attn_out"].reshape(L, H, S, Q).astype(np.float32)
        attns[:, b, :, o:o + Q, :] = a.transpose(0, 1, 3, 2)
    return x, attns


if __name__ == "__main__":
    _get_program()
    print("program built ok")
